# revision 1
# baseline (speedup 1.0000x reference)
"""Distributed GQA attention (B=2, S=2048, H=2048, 32 heads / 8 KV heads,
RoPE, causal) on 8 TRN2 NeuronCores.

Sharding: core c -> (batch b = c//4, head-group hg = c%4).
Each core computes q-heads [8hg, 8hg+8) and kv-heads [2hg, 2hg+2) of its
batch, runs attention locally (GQA groups stay on-core), then the four
cores of a batch AllGather their attention outputs (bf16) and each
computes a disjoint 512-column slice of the output projection, so no
all-reduce is needed.  Host reassembles the 8 disjoint slices.

Device layouts are transposed ([channel, row]) so RoPE / QK / AV / O-proj
all contract along partitions; softmax runs without max-subtraction
(scores are bounded: |q.k|/8 < ~40 << 88) and denominators come for free
from a ones-column appended to V in the AV matmul.
"""
import os
import sys

sys.path.insert(0, "/opt/trn_rl_repo")

import numpy as np
import ml_dtypes

import concourse.bass as bass
import concourse.mybir as mybir
import concourse.tile as tile
from concourse import bacc
from concourse import bass_utils

BF16 = mybir.dt.bfloat16
F32 = mybir.dt.float32
ADD = mybir.AluOpType.add
MULT = mybir.AluOpType.mult

B, S, H = 2, 2048, 2048
NH, NKV, HD = 32, 8, 64
SCALE = HD ** -0.5
RG = [[0, 1, 2, 3], [4, 5, 6, 7]]
N_CORES = 8
NT = S // 128          # 16 seq tiles
HT = H // 128          # 16 hidden tiles

TRACE = os.environ.get("BASS_KERNEL_TRACE", "0") == "1"
LAST_EXEC_NS = None
_COMPILED = None


def _install_profile_shim():
    import types
    try:
        from trn_agent_boot.trn_boot import _ntff_profile_via_ctypes
    except ImportError:
        return
    hook = _ntff_profile_via_ctypes("/opt/axon/libaxon_pjrt.so")
    mod = types.ModuleType("antenv.axon_hooks")
    mod.get_axon_ntff_profile_hook = lambda: hook
    mod.set_axon_ntff_profile_hook = lambda h: None
    sys.modules["antenv.axon_hooks"] = mod
    bass_utils.upload_artifacts = lambda tmpdir: tmpdir


def _build():
    nc = bacc.Bacc("TRN2", target_bir_lowering=False, debug=False,
                   num_devices=N_CORES)

    xt = nc.dram_tensor("xt", [H, S], BF16, kind="ExternalInput")
    wqt = nc.dram_tensor("wqt", [H, 512], BF16, kind="ExternalInput")
    wkt = nc.dram_tensor("wkt", [H, 128], BF16, kind="ExternalInput")
    wvt = nc.dram_tensor("wvt", [H, 128], BF16, kind="ExternalInput")
    wot = nc.dram_tensor("wot", [H, 512], BF16, kind="ExternalInput")
    bq = nc.dram_tensor("bq", [512, 1], F32, kind="ExternalInput")
    bk = nc.dram_tensor("bk", [128, 1], F32, kind="ExternalInput")
    bvh = nc.dram_tensor("bvh", [64, 8], F32, kind="ExternalInput")
    bo = nc.dram_tensor("bo", [512, 1], F32, kind="ExternalInput")
    qcos = nc.dram_tensor("qcos", [128, S], BF16, kind="ExternalInput")
    qsin = nc.dram_tensor("qsin", [128, S], BF16, kind="ExternalInput")
    kcos = nc.dram_tensor("kcos", [128, S], BF16, kind="ExternalInput")
    ksin = nc.dram_tensor("ksin", [128, S], BF16, kind="ExternalInput")
    maskd = nc.dram_tensor("maskd", [128, S], F32, kind="ExternalInput")
    out = nc.dram_tensor("out", [512, S], F32, kind="ExternalOutput")

    Exp = mybir.ActivationFunctionType.Exp

    from contextlib import ExitStack
    with tile.TileContext(nc) as tc:
        with ExitStack() as stk:
            ep = stk.enter_context
            big = ep(tc.tile_pool(name="big", bufs=16))     # xt / gathered o
            wpool = ep(tc.tile_pool(name="w", bufs=16))     # wqt / wot
            wkpool = ep(tc.tile_pool(name="wk", bufs=16))
            wvpool = ep(tc.tile_pool(name="wv", bufs=16))
            qpool = ep(tc.tile_pool(name="qt", bufs=4))
            kpool = ep(tc.tile_pool(name="kt", bufs=2))
            vpool = ep(tc.tile_pool(name="vv", bufs=16))
            opool = ep(tc.tile_pool(name="ot", bufs=4))
            tabpool = ep(tc.tile_pool(name="tab", bufs=4))
            mkpool = ep(tc.tile_pool(name="mk", bufs=1))
            ropepool = ep(tc.tile_pool(name="rope", bufs=6))
            expool = ep(tc.tile_pool(name="exp", bufs=4))
            nrmpool = ep(tc.tile_pool(name="nrm", bufs=2))
            ypool = ep(tc.tile_pool(name="yy", bufs=2))
            bpool = ep(tc.tile_pool(name="bias", bufs=12))
            pp = ep(tc.tile_pool(name="pp", bufs=2, space="PSUM"))
            scp = ep(tc.tile_pool(name="sc", bufs=3, space="PSUM"))
            avp = ep(tc.tile_pool(name="av", bufs=3, space="PSUM"))
            dram = ep(tc.tile_pool(name="dram", bufs=1, space="DRAM"))
            # ---------- input loads ----------
            xt_sb = []
            for t in range(HT):
                x_t = big.tile([128, S], BF16, name=f"xt{t}", tag="big")
                nc.sync.dma_start(out=x_t[:, :], in_=xt[128 * t:128 * (t + 1), :])
                xt_sb.append(x_t)
            wk_sb, wv_sb = [], []
            for t in range(HT):
                k_t = wkpool.tile([128, 128], BF16, name=f"wk{t}", tag="wk")
                nc.sync.dma_start(out=k_t[:, :], in_=wkt[128 * t:128 * (t + 1), :])
                wk_sb.append(k_t)
                v_t = wvpool.tile([128, 128], BF16, name=f"wv{t}", tag="wv")
                nc.sync.dma_start(out=v_t[:, :], in_=wvt[128 * t:128 * (t + 1), :])
                wv_sb.append(v_t)
            kcos_sb = tabpool.tile([128, S], BF16, name="kcos", tag="tab")
            nc.sync.dma_start(out=kcos_sb[:, :], in_=kcos[:, :])
            ksin_sb = tabpool.tile([128, S], BF16, name="ksin", tag="tab")
            nc.sync.dma_start(out=ksin_sb[:, :], in_=ksin[:, :])
            qcos_sb = tabpool.tile([128, S], BF16, name="qcos", tag="tab")
            nc.sync.dma_start(out=qcos_sb[:, :], in_=qcos[:, :])
            qsin_sb = tabpool.tile([128, S], BF16, name="qsin", tag="tab")
            nc.sync.dma_start(out=qsin_sb[:, :], in_=qsin[:, :])
            wq_sb = []
            for t in range(HT):
                q_t = wpool.tile([128, 512], BF16, name=f"wq{t}", tag="w")
                nc.sync.dma_start(out=q_t[:, :], in_=wqt[128 * t:128 * (t + 1), :])
                wq_sb.append(q_t)
            maskd_sb = mkpool.tile([128, S], F32, name="maskd", tag="mk")
            nc.sync.dma_start(out=maskd_sb[:, :], in_=maskd[:, :])
            bq_sb, bo_sb = [], []
            for o in range(4):
                b_t = bpool.tile([128, 1], F32, name=f"bq{o}", tag="bias")
                nc.sync.dma_start(out=b_t[:, :], in_=bq[128 * o:128 * (o + 1), :])
                bq_sb.append(b_t)
            bk_sb = bpool.tile([128, 1], F32, name="bk", tag="bias")
            nc.sync.dma_start(out=bk_sb[:, :], in_=bk[:, :])
            bvh_sb = bpool.tile([64, 8], F32, name="bvh", tag="bvh")
            nc.sync.dma_start(out=bvh_sb[:, :], in_=bvh[:, :])
            for o in range(4):
                b_t = bpool.tile([128, 1], F32, name=f"bo{o}", tag="bias")
                nc.sync.dma_start(out=b_t[:, :], in_=bo[128 * o:128 * (o + 1), :])
                bo_sb.append(b_t)

            def rope(psum, bias_ap, cos_sb, sin_sb, c, out_ap):
                """out = (psum+bias)*cos + shift32((psum+bias)*sin_pre)."""
                cs = slice(512 * c, 512 * (c + 1))
                tcos = ropepool.tile([128, 512], F32, name="tcos", tag="rope")
                nc.vector.scalar_tensor_tensor(
                    tcos[:, :], psum[:, :], bias_ap, cos_sb[:, cs],
                    op0=ADD, op1=MULT)
                tsin = ropepool.tile([128, 512], F32, name="tsin", tag="rope")
                nc.vector.scalar_tensor_tensor(
                    tsin[:, :], psum[:, :], bias_ap, sin_sb[:, cs],
                    op0=ADD, op1=MULT)
                tsh = ropepool.tile([128, 512], F32, name="tsh", tag="rope")
                for d, s in ((0, 32), (32, 0), (64, 96), (96, 64)):
                    nc.sync.dma_start(out=tsh[d:d + 32, :], in_=tsin[s:s + 32, :])
                nc.vector.tensor_tensor(out_ap, tcos[:, :], tsh[:, :], ADD)

            # ---------- K projection + rope ----------
            kT_sb = kpool.tile([128, S], BF16, name="kT", tag="kt")
            kT_sw = kpool.tile([128, S], BF16, name="kTswap", tag="kt")
            for c in range(4):
                ps = pp.tile([128, 512], F32, name="psk", tag="pp")
                for t in range(HT):
                    nc.tensor.matmul(ps[:, :], wk_sb[t][:, :],
                                     xt_sb[t][:, 512 * c:512 * (c + 1)],
                                     start=(t == 0), stop=(t == HT - 1))
                rope(ps, bk_sb[:, :], kcos_sb, ksin_sb, c,
                     kT_sb[:, 512 * c:512 * (c + 1)])
            # kT_sw: swapped kv halves (kv1 on partitions 0:64, kv0 on 64:128)
            nc.sync.dma_start(out=kT_sw[0:64, :], in_=kT_sb[64:128, :])
            nc.sync.dma_start(out=kT_sw[64:128, :], in_=kT_sb[0:64, :])

            # ---------- V projection (layout [rows, oc], 65-strided + ones) ----------
            v_sb = []
            for rt in range(NT):
                v_t = vpool.tile([128, 130], BF16, name=f"v{rt}", tag="v")
                nc.vector.memset(
                    v_t[:, :].rearrange("p (m c) -> p m c", c=65)[:, :, 64:65], 1.0)
                ps = pp.tile([128, 128], F32, name="psv", tag="pp")
                for t in range(HT):
                    nc.tensor.matmul(ps[:, :],
                                     xt_sb[t][:, 128 * rt:128 * (rt + 1)],
                                     wv_sb[t][:, :],
                                     start=(t == 0), stop=(t == HT - 1))
                nc.vector.tensor_copy(
                    v_t[:, :].rearrange("p (m c) -> p m c", c=65)[:, :, 0:64],
                    ps[:, :].rearrange("p (m c) -> p m c", c=64))
                v_sb.append(v_t)

            # ---------- Q projection + rope (scale folded into tables) ----------
            qT_sb = []
            for o in range(4):
                q_t = qpool.tile([128, S], BF16, name=f"qT{o}", tag="qt")
                qT_sb.append(q_t)
                for c in range(4):
                    ps = pp.tile([128, 512], F32, name="psq", tag="pp")
                    for t in range(HT):
                        nc.tensor.matmul(ps[:, :],
                                         wq_sb[t][:, 128 * o:128 * (o + 1)],
                                         xt_sb[t][:, 512 * c:512 * (c + 1)],
                                         start=(t == 0), stop=(t == HT - 1))
                    rope(ps, bq_sb[o][:, :], qcos_sb, qsin_sb, c,
                         q_t[:, 512 * c:512 * (c + 1)])

            # ---------- attention ----------
            oT_sb = []
            for j in range(4):
                o_t = opool.tile([128, S], BF16, name=f"oT{j}", tag="ot")
                oT_sb.append(o_t)

            bounce = [dram.tile([512, 1024], BF16, name=f"bounce{g}")
                      for g in range(2)]
            gath = [dram.tile([4, 512, 1024], BF16, name=f"gath{g}")
                    for g in range(2)]
            wo_sb = [None] * HT

            def emit_gather(g):
                for jj in range(4):
                    nc.sync.dma_start(
                        out=bounce[g][128 * jj:128 * (jj + 1), :],
                        in_=oT_sb[jj][:, 1024 * g:1024 * (g + 1)])
                nc.gpsimd.collective_compute(
                    "AllGather", mybir.AluOpType.bypass, replica_groups=RG,
                    ins=[bounce[g][:, :].opt()],
                    outs=[gath[g][:, :, :].opt()])

            def emit_oproj(g):
                gview = gath[g][:, :, :].rearrange("g i q -> (g i) q")
                gsb = []
                for t in range(HT):
                    g_t = big.tile([128, 1024], BF16, name=f"g{g}_{t}", tag="big")
                    nc.sync.dma_start(out=g_t[:, :],
                                      in_=gview[128 * t:128 * (t + 1), :])
                    gsb.append(g_t)
                if wo_sb[0] is None:
                    for t in range(HT):
                        w_t = wpool.tile([128, 512], BF16, name=f"wo{t}", tag="w")
                        nc.sync.dma_start(
                            out=w_t[:, :], in_=wot[128 * t:128 * (t + 1), :])
                        wo_sb[t] = w_t
                for o in range(4):
                    for qc in range(2):
                        ps = pp.tile([128, 512], F32, name="psy", tag="pp")
                        for t in range(HT):
                            nc.tensor.matmul(
                                ps[:, :], wo_sb[t][:, 128 * o:128 * (o + 1)],
                                gsb[t][:, 512 * qc:512 * (qc + 1)],
                                start=(t == 0), stop=(t == HT - 1))
                        y_t = ypool.tile([128, 512], F32, name="y", tag="y")
                        nc.vector.tensor_scalar_add(y_t[:, :], ps[:, :],
                                                    bo_sb[o][:, :])
                        nc.sync.dma_start(
                            out=out[128 * o:128 * (o + 1),
                                    1024 * g + 512 * qc:1024 * g + 512 * (qc + 1)],
                            in_=y_t[:, :])

            av_tiles = {}
            for phase, j in [(ph, jj) for ph in range(2) for jj in range(4)]:
                for qt in range(8 * phase, 8 * phase + 8):
                    kvl = j // 2
                    ke = kT_sb if kvl == 0 else kT_sw
                    ko = kT_sw if kvl == 0 else kT_sb
                    qs = slice(128 * qt, 128 * (qt + 1))
                    if qt % 4 == 0:
                        av_tiles[(j, 0)] = avp.tile([65, 512], F32,
                                                    name=f"av{j}e", tag="av")
                        av_tiles[(j, 1)] = avp.tile([65, 512], F32,
                                                    name=f"av{j}o", tag="av")
                    ave = av_tiles[(j, 0)]
                    avo = av_tiles[(j, 1)]
                    avs = slice(128 * (qt % 4), 128 * (qt % 4 + 1))
                    sce = sco = None
                    for kt in range(qt + 1):
                        c = kt % 4
                        if c == 0:
                            sce = scp.tile([128, 512], F32, name="sce", tag="sc")
                            sco = scp.tile([128, 512], F32, name="sco", tag="sc")
                        cs = slice(128 * c, 128 * (c + 1))
                        ks = slice(128 * kt, 128 * (kt + 1))
                        nc.tensor.matmul(sce[:, cs], ke[0:64, ks],
                                         qT_sb[j][0:64, qs],
                                         start=True, stop=True,
                                         tile_position=(0, 0))
                        nc.tensor.matmul(sco[:, cs], ko[64:128, ks],
                                         qT_sb[j][64:128, qs],
                                         start=True, stop=True,
                                         tile_position=(64, 0))
                        if kt == qt:
                            nc.vector.tensor_tensor(sce[:, cs], sce[:, cs],
                                                    maskd_sb[:, qs], ADD)
                            nc.vector.tensor_tensor(sco[:, cs], sco[:, cs],
                                                    maskd_sb[:, qs], ADD)
                        if c == 3 or kt == qt:
                            w = 128 * (c + 1)
                            ebuf = expool.tile([128, 512], BF16, name="ebuf",
                                               tag="exp")
                            obuf = expool.tile([128, 512], BF16, name="obuf",
                                               tag="exp")
                            nc.scalar.activation(ebuf[:, 0:w], sce[:, 0:w], Exp)
                            nc.scalar.activation(obuf[:, 0:w], sco[:, 0:w], Exp)
                            for cc in range(c + 1):
                                ktt = kt - c + cc
                                vs = v_sb[ktt][:, 65 * kvl:65 * kvl + 65]
                                ccs = slice(128 * cc, 128 * (cc + 1))
                                nc.tensor.matmul(ave[:, avs], vs, ebuf[:, ccs],
                                                 start=(ktt == 0),
                                                 stop=(ktt == qt))
                                nc.tensor.matmul(avo[:, avs], vs, obuf[:, ccs],
                                                 start=(ktt == 0),
                                                 stop=(ktt == qt))
                    if qt % 4 == 3:
                        qq = qt // 4
                        ocs = slice(512 * qq, 512 * (qq + 1))
                        for par in range(2):
                            avt = av_tiles[(j, par)]
                            hl = 2 * j + par
                            recip = nrmpool.tile([1, 512], F32, name="recip",
                                                 tag="recip")
                            nc.vector.reciprocal(recip[:, :], avt[64:65, :])
                            bcast = nrmpool.tile([64, 512], F32, name="bcast",
                                                 tag="bcast")
                            nc.gpsimd.partition_broadcast(bcast[:, :],
                                                          recip[:, :],
                                                          channels=64)
                            tsb = nrmpool.tile([64, 512], F32, name="tsb",
                                               tag="tsb")
                            nc.vector.tensor_tensor(tsb[:, :], avt[0:64, :],
                                                    bcast[:, :], MULT)
                            if par == 0:
                                nc.vector.tensor_scalar_add(
                                    oT_sb[j][0:64, ocs], tsb[:, :],
                                    bvh_sb[:, hl:hl + 1])
                            else:
                                for hh in range(2):
                                    nc.vector.tensor_scalar_add(
                                        oT_sb[j][64 + 32 * hh:96 + 32 * hh, ocs],
                                        tsb[32 * hh:32 * (hh + 1), :],
                                        bvh_sb[32 * hh:32 * (hh + 1),
                                               hl:hl + 1])
                if j == 3:
                    emit_gather(phase)
                    emit_oproj(phase)

    nc.compile()
    return nc


def kernel(**inputs):
    global _COMPILED, LAST_EXEC_NS
    x = np.asarray(inputs["hidden_states"], dtype=np.float32)
    mask = np.asarray(inputs["attention_mask"], dtype=np.float32)
    pos = np.asarray(inputs["position_ids"])
    Wq = np.asarray(inputs["Wq"], dtype=np.float32)
    bq = np.asarray(inputs["bq"], dtype=np.float32)
    Wk = np.asarray(inputs["Wk"], dtype=np.float32)
    bk = np.asarray(inputs["bk"], dtype=np.float32)
    Wv = np.asarray(inputs["Wv"], dtype=np.float32)
    bv = np.asarray(inputs["bv"], dtype=np.float32)
    Wo = np.asarray(inputs["Wo"], dtype=np.float32)
    bo = np.asarray(inputs["bo"], dtype=np.float32)

    bf = ml_dtypes.bfloat16
    # rope tables (from the position_ids input)
    p = pos[0].astype(np.float32)
    inv = 1.0 / (10000.0 ** (np.arange(0, HD, 2, dtype=np.float32) / HD))
    fr = p[:, None] * inv[None, :]                       # (S, 32)
    emb = np.concatenate([fr, fr], axis=1)               # (S, 64)
    cosT = np.cos(emb).T.astype(np.float32)              # (64, S)
    sinT = np.sin(emb).T.astype(np.float32)
    # pre-shifted signed sin: multiplied at src rows, then shifted to dst
    ss_pre = np.concatenate([sinT[32:64], -sinT[0:32]], axis=0)  # (64, S)
    kcos = np.tile(cosT, (2, 1)).astype(bf)
    ksin = np.tile(ss_pre, (2, 1)).astype(bf)
    qcos = (np.tile(cosT, (2, 1)) * SCALE).astype(bf)
    qsin = (np.tile(ss_pre, (2, 1)) * SCALE).astype(bf)

    maskd = np.empty((128, S), dtype=np.float32)
    for t in range(NT):
        maskd[:, 128 * t:128 * (t + 1)] = \
            mask[0, 0, 128 * t:128 * (t + 1), 128 * t:128 * (t + 1)].T

    in_maps = []
    for c in range(N_CORES):
        b, hg = c // 4, c % 4
        bv_slice = bv[128 * hg:128 * (hg + 1)]           # 2 kv heads x 64
        bvh = np.empty((64, 8), dtype=np.float32)
        for l in range(8):
            bvh[:, l] = bv_slice[64 * (l // 4):64 * (l // 4) + 64]
        in_maps.append({
            "xt": np.ascontiguousarray(x[b].T).astype(bf),
            "wqt": np.ascontiguousarray(Wq[512 * hg:512 * (hg + 1), :].T).astype(bf),
            "wkt": np.ascontiguousarray(Wk[128 * hg:128 * (hg + 1), :].T).astype(bf),
            "wvt": np.ascontiguousarray(Wv[128 * hg:128 * (hg + 1), :].T).astype(bf),
            "wot": np.ascontiguousarray(Wo[512 * hg:512 * (hg + 1), :].T).astype(bf),
            "bq": np.ascontiguousarray(bq[512 * hg:512 * (hg + 1)])[:, None],
            "bk": np.ascontiguousarray(bk[128 * hg:128 * (hg + 1)])[:, None],
            "bvh": bvh,
            "bo": np.ascontiguousarray(bo[512 * hg:512 * (hg + 1)])[:, None],
            "qcos": qcos, "qsin": qsin, "kcos": kcos, "ksin": ksin,
            "maskd": maskd,
        })

    if _COMPILED is None:
        _install_profile_shim()
        _COMPILED = _build()

    res = bass_utils.run_bass_kernel_spmd(
        _COMPILED, in_maps, core_ids=list(range(N_CORES)), trace=TRACE)
    LAST_EXEC_NS = res.exec_time_ns

    outb = []
    for b in range(B):
        yt = np.concatenate([res.results[4 * b + hg]["out"]
                             for hg in range(4)], axis=0)   # [2048 oc, 2048 q]
        outb.append(yt.T)
    return np.stack(outb).astype(np.float32)

